# revision 8
# baseline (speedup 1.0000x reference)
"""Trainium2 Bass kernel for batched int8 matmul with f32 dequant epilogue.

Computes: out[b,m,n] = (sum_k a[b,m,k] * b[b,k,n]) * alpha   (int8 x int8,
int32-exact accumulation via bf16 PE matmuls into fp32 PSUM).

Sharding: batch dim B=16 is split across 8 NeuronCores (2 batches/core,
data parallel, no communication).

Host-side prep per core: a-shard is transposed to [B_PER_CORE, K, M] and cast
to bf16 (exact for int8 values); b-shard stays int8 and is cast to bf16
in-flight by SWDGE casting DMAs on-device.
"""

import sys

try:  # noqa: SIM105
    import concourse.bass  # noqa: F401
except ImportError:
    sys.path.insert(0, "/opt/trn_rl_repo")

from contextlib import ExitStack

import ml_dtypes
import numpy as np

import concourse.bass as bass  # noqa: F401  (kept for API parity)
import concourse.tile as tile
from concourse import bacc, mybir
from concourse.bass_utils import run_bass_kernel_spmd


def _ensure_axon_hooks_stub():
    """bass_utils imports antenv.axon_hooks when tracing is requested (e.g.
    via a BASS_TRACE env); this agent image ships antenv without that
    submodule, so provide a no-op stub to keep the graceful fallback."""
    try:
        import antenv.axon_hooks  # noqa: F401
    except ImportError:
        import types

        mod = types.ModuleType("antenv.axon_hooks")
        mod.get_axon_ntff_profile_hook = lambda: None
        mod.set_axon_ntff_profile_hook = lambda h: None
        sys.modules["antenv.axon_hooks"] = mod


_ensure_axon_hooks_stub()

N_CORES = 8
B, M, K, N = 16, 1024, 4096, 4096
B_PER_CORE = B // N_CORES

KT, MT, NT = 128, 128, 512  # k / m / n tile sizes
K_TILES = K // KT  # 32
M_TILES = M // MT  # 8
N_TILES = N // NT  # 8
B_CHUNK = 8  # k-tiles per B-matrix casting DMA


def _build(alpha: float):
    nc = bacc.Bacc(
        "TRN2",
        target_bir_lowering=False,
        debug=False,
        num_devices=N_CORES,
    )
    aT = nc.declare_dram_parameter(
        "aT", [B_PER_CORE, K, M], mybir.dt.bfloat16, isOutput=False
    )
    b = nc.declare_dram_parameter(
        "b", [B_PER_CORE, K, N], mybir.dt.int8, isOutput=False
    )
    out = nc.declare_dram_parameter(
        "out", [B_PER_CORE, M, N], mybir.dt.float32, isOutput=True
    )

    with tile.TileContext(nc) as tc, ExitStack() as ctx:
        a_pool = ctx.enter_context(tc.tile_pool(name="a_pool", bufs=2 * K_TILES))
        b_pool = ctx.enter_context(tc.tile_pool(name="b_pool", bufs=6))
        o_pool = ctx.enter_context(tc.tile_pool(name="o_pool", bufs=4))
        p_pool = ctx.enter_context(tc.tile_pool(name="psum", bufs=6, space="PSUM"))

        for bi in range(B_PER_CORE):
            a_tiles = []
            for kt in range(K_TILES):
                at = a_pool.tile([KT, M], mybir.dt.bfloat16, tag="aT")
                # Alternate HWDGE rings (SP / ACT) to double issue rate.
                eng = nc.sync if kt % 2 == 0 else nc.scalar
                eng.dma_start(at[:], aT[bi, kt * KT : (kt + 1) * KT, :])
                a_tiles.append(at)

            for nb in range(N_TILES):
                # Smaller leading chunks on the very first slab so the first
                # matmuls start as soon as possible.
                chunk_sizes = (
                    [2, 6, 8, 8, 8] if (bi == 0 and nb == 0) else [8, 8, 8, 8]
                )
                b_tiles = []  # (k_tile_start, n_ktiles, tile)
                k0 = 0
                for csz in chunk_sizes:
                    bt = b_pool.tile([KT, B_CHUNK * NT], mybir.dt.bfloat16, tag="b")
                    src = b[
                        bi,
                        k0 * KT : (k0 + csz) * KT,
                        nb * NT : (nb + 1) * NT,
                    ].rearrange("(t p) n -> p t n", p=KT)
                    dst = bt[:, : csz * NT].rearrange("p (t n) -> p t n", n=NT)
                    nc.gpsimd.dma_start(dst, src)  # int8 -> bf16 casting DMA
                    b_tiles.append((k0, csz, bt))
                    k0 += csz

                for mt in range(M_TILES):
                    ps = p_pool.tile([MT, NT], mybir.dt.float32, tag="ps")
                    for k0, csz, bt in b_tiles:
                        for off in range(csz):
                            kt = k0 + off
                            nc.tensor.matmul(
                                ps[:],
                                a_tiles[kt][:, mt * MT : (mt + 1) * MT],
                                bt[:, off * NT : (off + 1) * NT],
                                start=(kt == 0),
                                stop=(kt == K_TILES - 1),
                            )
                    ot = o_pool.tile([MT, NT], mybir.dt.float32, tag="o")
                    nc.vector.tensor_scalar_mul(ot[:], ps[:], alpha)
                    # Stores go on the ACT HWDGE ring so batch N+1's A-tile
                    # loads (SP ring) don't queue behind them.
                    nc.scalar.dma_start(
                        out[bi, mt * MT : (mt + 1) * MT, nb * NT : (nb + 1) * NT],
                        ot[:],
                    )
    nc.compile()
    return nc


def run(a, b, alpha, trace: bool = False, **spmd_kwargs):
    a = np.asarray(a)
    b = np.asarray(b)
    if a.dtype != np.int8:
        a = a.astype(np.int8)
    if b.dtype != np.int8:
        b = b.astype(np.int8)

    nc = _build(float(alpha))

    in_maps = []
    for i in range(N_CORES):
        a_sh = a[i * B_PER_CORE : (i + 1) * B_PER_CORE]
        b_sh = np.ascontiguousarray(b[i * B_PER_CORE : (i + 1) * B_PER_CORE])
        aT = a_sh.transpose(0, 2, 1).astype(ml_dtypes.bfloat16)
        in_maps.append({"aT": aT, "b": b_sh})

    res = run_bass_kernel_spmd(
        nc, in_maps, list(range(N_CORES)), trace=trace, **spmd_kwargs
    )
    full = np.concatenate([r["out"] for r in res.results], axis=0)
    return full, res


def kernel(a, b, alpha):
    full, _ = run(a, b, alpha)
    return full
